# revision 26
# baseline (speedup 1.0000x reference)
"""Trainium2 Bass kernel for nn_AttLSTM (attention-LSTM, K=4 steps).

Math per step (reference):
    a = softmax(h @ g_S.T, axis=1)            # [B, S]
    r = a @ g_S                               # [B, D]
    gates = f_x @ W_ih.T + b_ih + [h, r] @ W_hh.T + b_hh
    i, f, g, o = split(gates, 4)
    c' = sig(f)*c + sig(i)*tanh(g); h' = sig(o)*tanh(c') + f_x

Design (per core, data-parallel over batch: B_loc = 512 rows/core):
  - logits matmuls in fp16 (fp8 logits distort the peaked softmax too
    much: measured 4e-2 rel err vs the 2e-2 gate).  The attention
    readout r = p @ g AND the gates matmul run in fp8e4 with
    MatmulPerfMode.DoubleRow (2 k-tiles per instruction, ~1.7x PE rate;
    emulated rel err 1.4e-2 < 2e-2).
  - x @ W_ih.T + biases precomputed once (x == f_x every step) -> xw
    in fp16 (fp8 xw eats too much of the error budget).
  - g_S kept two ways, both RESIDENT in SBUF (no per-step DRAM
    streaming): transposed [D, S] fp16 (g_T, rhs of the logits matmul)
    and natural [S, D] fp8 (g8, rhs of the DoubleRow readout).
  - ALL transposes via PE transpose-mode in groups of [128,128] blocks
    into one PSUM bank + one strided copy back to SBUF.
  - softmax per 128-row b-tile: per-512-chunk negated max (DVE, from
    PSUM), exp with per-chunk bias straight from PSUM (ACT, fp8 out) +
    accum_out row-sums.  The global per-row rescale p *= exp(m_chunk -
    m_row) is folded INTO the p-transpose: the transpose matmul's
    "identity" operand is replaced by diag(fcorr) per chunk, so the
    rescale costs zero extra DVE/ACT passes.  The denominator uses the
    same fp8-quantized fcorr for consistency.
  - logits matmuls emitted in PSUM-bank *quads* with the contraction
    loop outermost so 4 consecutive instructions share lhsT.
  - prolog: W_ih/W_hh/f_x loaded raw f32 on the sync+scalar HW DMA
    queues (parallel rings), g_S cast-loaded f32->f16 on the gpsimd SW
    queues; gpsimd casts staging tiles to f16/fp8 SBUF-to-SBUF.  The
    step-0 A-phase of b-tiles 0/1 is threaded between the g_S group
    transposes so the PE never idles waiting for DMA.
  - sigmoid computed as 0.5*tanh(x/2)+0.5 so the single `exp_and_others`
    ACT table set (Exp + Tanh) serves the whole kernel.
  - LSTM pointwise math split between DVE and gpsimd (gpsimd is
    otherwise idle), carrying z = 2c as state.
"""

import os
import sys

import numpy as np

for _p in ("/opt/trn_rl_repo",):
    if _p not in sys.path and os.path.isdir(_p):
        sys.path.insert(0, _p)

# Problem sizes (hardcoded per spec).
B, S, D = 4096, 8192, 512
H = D
N_CORES = 8
B_LOC = B // N_CORES          # 512 rows per core
K_STEPS = 4
P = 128                       # partitions


def build_bass(b_loc=B_LOC, s=S, k_steps=K_STEPS):
    import concourse.mybir as mybir
    import concourse.tile as tile
    from concourse import bacc
    from concourse.masks import make_identity
    from contextlib import ExitStack

    f32 = mybir.dt.float32
    f16 = mybir.dt.float16
    f8 = mybir.dt.float8e4
    AF = mybir.ActivationFunctionType
    ALU = mybir.AluOpType
    AX = mybir.AxisListType
    DR = mybir.MatmulPerfMode.DoubleRow

    nb = b_loc // P               # b-tiles per core
    nd = D // P                   # contraction chunks over D
    ns = s // 512                 # s-chunks of 512
    ns2 = s // 1024               # softmax pair-chunks of 1024
    nt = s // P                   # s-tiles of 128
    ng = (4 * H) // 512           # gate chunks

    nc = bacc.Bacc("TRN2", target_bir_lowering=False, debug=False)

    f_x = nc.dram_tensor("f_x", [b_loc, D], f32, kind="ExternalInput")
    g_S = nc.dram_tensor("g_S", [s, D], f32, kind="ExternalInput")
    W_ih = nc.dram_tensor("W_ih", [4 * H, D], f32, kind="ExternalInput")
    W_hh = nc.dram_tensor("W_hh", [4 * H, 2 * H], f32, kind="ExternalInput")
    b_ih = nc.dram_tensor("b_ih", [4 * H], f32, kind="ExternalInput")
    b_hh = nc.dram_tensor("b_hh", [4 * H], f32, kind="ExternalInput")
    out = nc.dram_tensor("out", [b_loc, D], f32, kind="ExternalOutput")

    with tile.TileContext(nc) as tc, ExitStack() as ctx:
        const = ctx.enter_context(tc.tile_pool(name="const", bufs=1))
        g_T = const.tile([P, nd, s], f16)            # g_S.T resident fp16
        g8 = const.tile([P, nt, D], f8)              # g_S natural resident fp8
        whhT = const.tile([P, 2 * nd, 4 * H], f8)    # W_hh.T resident fp8
        xw = const.tile([P, nb, 4 * H], f16)         # f_x@W_ih.T + biases
        fx16 = const.tile([P, nb, D], f16)
        br16 = const.tile([1, 4 * H], f16)
        ones16 = const.tile([1, P], f16)
        ident = const.tile([P, P], f16)
        ident8 = const.tile([P, P], f8)

        # staging pool for raw f32 W chunks + f16 g groups
        wst_pool = ctx.enter_context(tc.tile_pool(name="wst", bufs=2))
        w16_pool = ctx.enter_context(tc.tile_pool(name="w16", bufs=3))
        wih_pool = ctx.enter_context(tc.tile_pool(name="wih", bufs=1))
        p_pool = ctx.enter_context(tc.tile_pool(name="p_pool", bufs=2))
        pt_pool = ctx.enter_context(tc.tile_pool(name="ptp", bufs=3))
        ht_pool = ctx.enter_context(tc.tile_pool(name="htp", bufs=4))
        h8_pool = ctx.enter_context(tc.tile_pool(name="h8p", bufs=4))
        rt_pool = ctx.enter_context(tc.tile_pool(name="rtp", bufs=1))
        rh_pool = ctx.enter_context(tc.tile_pool(name="rhp", bufs=2))
        lstm_pool = ctx.enter_context(tc.tile_pool(name="lstm", bufs=2))
        z_pool = ctx.enter_context(tc.tile_pool(name="zp", bufs=4))
        st_pool = ctx.enter_context(tc.tile_pool(name="stp", bufs=2))
        dg_pool = ctx.enter_context(tc.tile_pool(name="dgp", bufs=8))

        ps_log = ctx.enter_context(tc.tile_pool(name="ps_log", bufs=2, space="PSUM"))
        ps_g = ctx.enter_context(tc.tile_pool(name="ps_g", bufs=2, space="PSUM"))
        ps_tp = ctx.enter_context(tc.tile_pool(name="ps_tp", bufs=2, space="PSUM"))

        make_identity(nc, ident[:])
        make_identity(nc, ident8[:])

        _tpn = [0]

        def tp_group(blocks, dst, copy_engine="v", dst2=None,
                     copy2_engine="s"):
            """PE-transpose len(blocks) [128,128] fp16 blocks into one
            PSUM group tile, then one (possibly strided) copy into dst
            (shape [P, len(blocks), P]).  dst2 (if given) gets a second
            copy (e.g. fp8 cast of an fp16 transpose)."""
            n = len(blocks)
            _tpn[0] += 1
            tp = ps_tp.tile([P, n, P], f16, tag="tp", name=f"tp_{_tpn[0]}")
            for t, blk in enumerate(blocks):
                nc.tensor.transpose(tp[:, t, :], blk, ident[:])
            src = tp[:]
            if copy_engine == "v":
                nc.vector.tensor_copy(dst, src)
            else:
                nc.scalar.copy(dst, src)
            if dst2 is not None:
                if copy2_engine == "v":
                    nc.vector.tensor_copy(dst2, src)
                else:
                    nc.scalar.copy(dst2, src)

        # ---------------- prolog DMA kickoff ----------------
        nc.vector.memset(ones16[:], 1.0)

        nc.gpsimd.dma_start(br16[:], b_ih[:].rearrange("(a n) -> a n", a=1))
        nc.gpsimd.dma_start(br16[:], b_hh[:].rearrange("(a n) -> a n", a=1),
                            accum_op=ALU.add)

        # f_x cast-loaded f32->f16 on gpsimd (first in its queue)
        for j in range(nb):
            nc.gpsimd.dma_start(fx16[:, j, :], f_x[j * P:(j + 1) * P, :])

        # W_ih cast-loaded f32->f16 on gpsimd (chunk = 4 row-tiles =
        # one 512-wide gate-column chunk)
        wih16 = {}

        def load_wih(cchunk):
            t = w16_pool.tile([P, 4, D], f16, tag="g16", name=f"wih16_{cchunk}")
            nc.gpsimd.dma_start(
                t[:], W_ih[cchunk * 4 * P:(cchunk + 1) * 4 * P, :].rearrange(
                    "(a p) d -> p a d", p=P))
            wih16[cchunk] = t


        # W_hh raw f32 chunks on sync HW queues (chunk = 1 row-tile);
        # nothing else ever waits on the sync queue, so the staged-slot
        # reuse deps cannot deadlock other engines.
        whh_st = {}
        for hchunk in range(16):
            t = wst_pool.tile([P, 1, 2 * H], f32, tag="wst",
                              name=f"whhst_{hchunk}")
            nc.sync.dma_start(
                t[:], W_hh[hchunk * P:(hchunk + 1) * P, :].rearrange(
                    "(a p) d -> p a d", p=P))
            whh_st[hchunk] = t

        # g_S cast-loads f32->f16 on gpsimd SW queues (group = 4 s-tiles)
        def load_g(tg4):
            gt = w16_pool.tile([P, 4, D], f16, tag="g16", name=f"gload_{tg4}")
            nc.gpsimd.dma_start(
                gt[:], g_S[tg4 * 4 * P:(tg4 + 1) * 4 * P, :].rearrange(
                    "(a p) d -> p a d", p=P))
            return gt

        # ---------------- prolog compute helpers ----------------
        # f_x transposes: fp16 for logits lhsT + fp8 for gates lhsT
        hT, hT8 = {}, {}

        def emit_fxT():
            for j in range(nb):
                t = ht_pool.tile([P, nd, P], f16, tag="hT", name=f"fxT_{j}")
                t8 = h8_pool.tile([P, nd, P], f8, tag="hT8", name=f"fxT8_{j}")
                tp_group([fx16[:, j, kk * P:(kk + 1) * P] for kk in range(nd)],
                         t[:], copy_engine="v" if j % 2 == 0 else "s",
                         dst2=t8[:], copy2_engine="s" if j % 2 == 0 else "v")
                hT[j] = t
                hT8[j] = t8

        def emit_xw_chunk(cchunk):
            """W_ih rows [512c, 512(c+1)) -> wihT columns; then xw chunk c
            for all b-tiles.  Chunk c of xw only needs those columns."""
            w16 = wih16.pop(cchunk)
            wihT = wih_pool.tile([P, nd, 4 * P], f16, tag="wih",
                                 name=f"wihT_{cchunk}")
            for a in range(4):
                tp_group([w16[:, a, kk * P:(kk + 1) * P] for kk in range(nd)],
                         wihT[:, :, a * P:(a + 1) * P],
                         copy_engine="v" if a % 2 == 0 else "s")
            for j in range(nb):
                gp = ps_g.tile([P, 512], f32, tag="psg", name=f"xwps_{j}_{cchunk}")
                nc.tensor.matmul(gp[:], ones16[:],
                                 br16[:, cchunk * 512:(cchunk + 1) * 512],
                                 start=True, stop=False)
                for kk in range(nd):
                    nc.tensor.matmul(gp[:], hT[j][:, kk, :], wihT[:, kk, :],
                                     start=False, stop=(kk == nd - 1))
                nc.vector.tensor_copy(xw[:, j, cchunk * 512:(cchunk + 1) * 512],
                                      gp[:])

        def emit_whh_chunk(hchunk):
            """W_hh row-tile -> whhT fp8 columns [128h, 128(h+1))."""
            st = whh_st.pop(hchunk)
            w16 = w16_pool.tile([P, 1, 2 * H], f16, tag="wh16",
                                name=f"whh16_{hchunk}")
            if hchunk % 2 == 0:
                nc.scalar.copy(w16[:], st[:])
            else:
                nc.vector.tensor_copy(w16[:], st[:])
            tp_group([w16[:, 0, kk * P:(kk + 1) * P] for kk in range(2 * nd)],
                     whhT[:, :, hchunk * P:(hchunk + 1) * P],
                     copy_engine="v" if hchunk % 2 == 0 else "s")
            del st

        def emit_g(tg4, gt):
            # transposes first so their copies aren't queued behind the
            # fp8 cast; cast alternates DVE/ACT
            for a in range(4):
                t = tg4 * 4 + a
                tp_group([gt[:, a, kk * P:(kk + 1) * P] for kk in range(nd)],
                         g_T[:, :, t * P:(t + 1) * P],
                         copy_engine="v" if t % 2 == 0 else "s")
            if tg4 % 2 == 0:
                nc.vector.tensor_copy(g8[:, tg4 * 4:(tg4 + 1) * 4, :], gt[:])
            else:
                nc.scalar.copy(g8[:, tg4 * 4:(tg4 + 1) * 4, :], gt[:])

        # ---------------- step state ----------------
        z = {}
        for j in range(nb):
            zt = z_pool.tile([P, D], f16, tag="z", name=f"z0_{j}")
            nc.vector.memset(zt[:], 0.0)
            z[j] = zt

        pbuf, negmaxes, sums, fcorr, rsum = {}, {}, {}, {}, {}

        def alloc_A(j):
            pbuf[j] = p_pool.tile([P, s], f8, tag="p", name=f"p_{j}")
            negmaxes[j] = st_pool.tile([P, ns2], f32, tag="nmx", name=f"nmx_{j}")
            sums[j] = st_pool.tile([P, ns2], f32, tag="sums", name=f"sums_{j}")

        def emit_A_pair(j, pc):
            """logits + negmax + exp for the 1024-wide pair-chunk pc of
            b-tile j.  One [P,1024] psl tile spans 2 PSUM banks; the
            contraction loop is outermost so 2 consecutive matmuls share
            lhsT.  With bufs=2, pair pc+1's matmuls fully hide the exp of
            pair pc (which is the stats at 1024 granularity: half the
            DVE/ACT instruction count of 512-chunks)."""
            ps = ps_log.tile([P, 1024], f32, tag="psl", name=f"psl_{j}_{pc}")
            for kk in range(nd):
                for u in range(2):
                    nc.tensor.matmul(
                        ps[:, u * 512:(u + 1) * 512], hT[j][:, kk, :],
                        g_T[:, kk, (2 * pc + u) * 512:(2 * pc + u + 1) * 512],
                        start=(kk == 0), stop=(kk == nd - 1))
            nc.vector.tensor_reduce(
                negmaxes[j][:, pc:pc + 1], ps[:],
                axis=AX.X, op=ALU.max, negate=True)
            nc.scalar.activation(
                pbuf[j][:, pc * 1024:(pc + 1) * 1024], ps[:],
                AF.Exp, bias=negmaxes[j][:, pc:pc + 1],
                accum_out=sums[j][:, pc:pc + 1])

        def emit_A_quad(j, cq):
            emit_A_pair(j, 2 * cq)
            emit_A_pair(j, 2 * cq + 1)

        def emit_A(j):
            alloc_A(j)
            for pc in range(ns2):
                emit_A_pair(j, pc)

        def emit_fin(j):
            """global max, correction factors (quantized fp8 for the
            diag fold), 1/sum for b-tile j"""
            nm = st_pool.tile([P, 1], f32, tag="nm", name=f"nm_{j}")
            nc.vector.tensor_reduce(nm[:], negmaxes[j][:], axis=AX.X, op=ALU.min)
            delta = st_pool.tile([P, ns2], f32, tag="delta", name=f"delta_{j}")
            # delta_i = m_i - m = -negmax_i + nm
            nc.vector.tensor_scalar(delta[:], negmaxes[j][:], -1.0, nm[:],
                                    op0=ALU.mult, op1=ALU.add)
            fc = st_pool.tile([P, ns2], f32, tag="fc", name=f"fc_{j}")
            nc.scalar.activation(fc[:], delta[:], AF.Exp)
            fcorr[j] = fc
            # quantize fcorr exactly as the diag-fold will (f32 -> fp8 RNE)
            fc8 = st_pool.tile([P, ns2], f8, tag="fc8", name=f"fc8_{j}")
            nc.vector.tensor_copy(fc8[:], fc[:])
            fc8f = st_pool.tile([P, ns2], f32, tag="fc8f", name=f"fc8f_{j}")
            nc.scalar.copy(fc8f[:], fc8[:])
            ws = st_pool.tile([P, ns2], f32, tag="ws", name=f"ws_{j}")
            nc.vector.scalar_tensor_tensor(ws[:], sums[j][:], 0.0, fc8f[:],
                                           op0=ALU.add, op1=ALU.mult)
            ssum = st_pool.tile([P, 1], f32, tag="ssum", name=f"ssum_{j}")
            nc.vector.tensor_reduce(ssum[:], ws[:], axis=AX.X, op=ALU.add)
            rs = st_pool.tile([P, 1], f32, tag="rs", name=f"rs_{j}")
            nc.vector.reciprocal(rs[:], ssum[:])
            rsum[j] = rs

        def emit_B(j, k):
            """transpose p (with the fcorr diag fold), fp8 DR readout,
            fp8 DR gates, LSTM update"""
            rp = ps_g.tile([P, D], f32, tag="psg", name=f"psr_{j}")
            pTg = {}
            diags = {}

            def mk_diag(pc):
                dg = dg_pool.tile([P, P], f8, tag="dg", name=f"dg_{j}_{pc}")
                if pc % 2 == 0:
                    nc.vector.tensor_scalar_mul(dg[:], ident8[:],
                                                fcorr[j][:, pc:pc + 1])
                else:
                    nc.scalar.mul(dg[:], ident8[:], fcorr[j][:, pc:pc + 1])
                diags[pc] = dg

            def tpg(ig):
                """transpose-and-rescale: regular matmul p_blk.T @ diag(fc)
                (transpose-mode requires a permutation rhs, a plain matmul
                does not).  LDWEIGHTS = p block, rhs = diag, N=128."""
                grp = pt_pool.tile([P, 4, P], f8, tag="pt", name=f"pt_{j}_{ig}")
                _tpn[0] += 1
                tp = ps_tp.tile([P, 4, P], f32, tag="tp",
                                name=f"ptp_{_tpn[0]}")
                dg = diags.pop(ig // 2) if ig % 2 == 1 else diags[ig // 2]
                for t in range(4):
                    nc.tensor.matmul(
                        tp[:, t, :],
                        pbuf[j][:, (ig * 4 + t) * P:(ig * 4 + t + 1) * P],
                        dg[:], start=True, stop=True)
                if ig % 2 == 0:
                    nc.vector.tensor_copy(grp[:], tp[:])
                else:
                    nc.scalar.copy(grp[:], tp[:])
                pTg[ig] = grp

            mk_diag(0)
            mk_diag(1)
            mk_diag(2)
            tpg(0)
            tpg(1)
            for ig in range(nt // 4):
                if ig % 2 == 0 and ig // 2 + 3 < ns2:
                    mk_diag(ig // 2 + 3)
                if ig + 2 < nt // 4:
                    tpg(ig + 2)
                for u in range(2):
                    c = ig * 2 + u
                    nc.tensor.matmul(rp[:], pTg[ig][:, 2 * u:2 * u + 2, :],
                                     g8[:, ig * 4 + 2 * u:ig * 4 + 2 * u + 2, :],
                                     start=(c == 0), stop=(c == nt // 2 - 1),
                                     perf_mode=DR)
                del pTg[ig]
            r16 = rh_pool.tile([P, D], f16, tag="r16", bufs=1, name=f"r16_{j}")
            nc.vector.tensor_scalar_mul(r16[:], rp[:], rsum[j][:])
            rT8 = rt_pool.tile([P, nd, P], f8, tag="rT", name=f"rT_{j}")
            tp_group([r16[:, kk * P:(kk + 1) * P] for kk in range(nd)], rT8[:])
            # gates = xw + h@Whh_h.T + r@Whh_r.T, fp8 DoubleRow, in
            # PSUM-bank pairs with the contraction loop outermost
            tt = [None] * ng
            for half in range(2):
                gp = [ps_g.tile([P, 512], f32, tag="psg",
                                name=f"psg_{j}_{half}_{u}") for u in range(2)]
                for q in range(nd // 2):
                    for u in range(2):
                        n = half * 2 + u
                        nc.tensor.matmul(
                            gp[u][:], hT8[j][:, 2 * q:2 * q + 2, :],
                            whhT[:, 2 * q:2 * q + 2, n * 512:(n + 1) * 512],
                            start=(q == 0), stop=False, perf_mode=DR)
                for q in range(nd // 2):
                    for u in range(2):
                        n = half * 2 + u
                        nc.tensor.matmul(
                            gp[u][:], rT8[:, 2 * q:2 * q + 2, :],
                            whhT[:, nd + 2 * q:nd + 2 * q + 2,
                                 n * 512:(n + 1) * 512],
                            start=False, stop=(q == nd // 2 - 1), perf_mode=DR)
                for u in range(2):
                    n = half * 2 + u
                    pre = lstm_pool.tile([P, 512], f16, tag="pre", bufs=1,
                                         name=f"pre_{j}_{n}")
                    nc.vector.scalar_tensor_tensor(
                        pre[:], gp[u][:], 0.0, xw[:, j, n * 512:(n + 1) * 512],
                        op0=ALU.add, op1=ALU.add)
                    t = lstm_pool.tile([P, 512], f16, tag=f"t{n}", bufs=1,
                                       name=f"t{n}_{j}")
                    # i,f,o gates: tanh(x/2) (-> sigmoid); g gate: tanh(x)
                    nc.scalar.activation(t[:], pre[:], AF.Tanh,
                                         scale=1.0 if n == 2 else 0.5)
                    tt[n] = t
            ti, tf, tg, to = tt
            # z' = 0.5*(tf+1)*z + (ti+1)*tg       (z = 2c)
            v = lstm_pool.tile([P, D], f16, tag="v", bufs=1, name=f"v_{j}")
            nc.vector.scalar_tensor_tensor(v[:], ti[:], 1.0, tg[:],
                                           op0=ALU.add, op1=ALU.mult)
            q = lstm_pool.tile([P, D], f16, tag="q", bufs=1, name=f"q_{j}")
            nc.vector.scalar_tensor_tensor(q[:], tf[:], 1.0, z[j][:],
                                           op0=ALU.add, op1=ALU.mult)
            zn = z_pool.tile([P, D], f16, tag="z", name=f"z_{j}")
            nc.vector.scalar_tensor_tensor(zn[:], q[:], 0.5, v[:],
                                           op0=ALU.mult, op1=ALU.add)
            z[j] = zn
            # h' = 0.5*(to+1)*tanh(z'/2) + f_x
            y = lstm_pool.tile([P, D], f16, tag="y", bufs=1, name=f"y_{j}")
            nc.scalar.activation(y[:], zn[:], AF.Tanh, scale=0.5)
            w = lstm_pool.tile([P, D], f16, tag="w", bufs=1, name=f"w_{j}")
            nc.vector.scalar_tensor_tensor(w[:], to[:], 1.0, y[:],
                                           op0=ALU.add, op1=ALU.mult)
            if k < k_steps - 1:
                h16 = rh_pool.tile([P, D], f16, tag="h16", bufs=1, name=f"h16_{j}")
                nc.vector.scalar_tensor_tensor(h16[:], w[:], 0.5, fx16[:, j, :],
                                               op0=ALU.mult, op1=ALU.add)
                hTn = ht_pool.tile([P, nd, P], f16, tag="hT", name=f"hT_{j}")
                hTn8 = h8_pool.tile([P, nd, P], f8, tag="hT8", name=f"hT8_{j}")
                tp_group([h16[:, kk * P:(kk + 1) * P] for kk in range(nd)],
                         hTn[:], copy_engine="v" if j % 2 == 0 else "s",
                         dst2=hTn8[:], copy2_engine="s" if j % 2 == 0 else "v")
                hT[j] = hTn
                hT8[j] = hTn8
            else:
                ho = z_pool.tile([P, D], f16, tag="z", name=f"ho_{j}")
                nc.vector.scalar_tensor_tensor(ho[:], w[:], 0.5, fx16[:, j, :],
                                               op0=ALU.mult, op1=ALU.add)
                nc.gpsimd.dma_start(out[j * P:(j + 1) * P, :], ho[:])

        # ---------------- interleaved prolog + step-0 A(0)/A(1) --------
        # gpsimd FIFO rule: a load that reuses a g16-ring slot freed by a
        # gpsimd cast must be emitted AFTER that cast (FIFO self-waits
        # deadlock otherwise).  Ring (bufs=3) allocation order:
        # wih0 wih1 g0 | g1 g2 g3 | wih2 wih3 g4 | g5 g6 g7 | g8 ...
        load_wih(0)
        load_wih(1)
        gts = {0: load_g(0)}
        emit_fxT()
        emit_xw_chunk(0)            # frees wih0 (PE-side)
        gts[1] = load_g(1)          # slot of wih0
        emit_xw_chunk(1)            # frees wih1
        gts[2] = load_g(2)          # slot of wih1
        # pbufs share p_pool slots; allocate after the early prolog tiles
        alloc_A(0)
        if nb > 1:
            alloc_A(1)
        emit_g(0, gts.pop(0))
        gts[3] = load_g(3)          # slot of g0: after cast g0
        emit_g(1, gts.pop(1))
        load_wih(2)                 # slot of g1: after cast g1
        emit_g(2, gts.pop(2))
        load_wih(3)                 # slot of g2: after cast g2
        emit_xw_chunk(2)
        emit_g(3, gts.pop(3))
        gts[4] = load_g(4)          # slot of g3: after cast g3
        emit_A_quad(0, 0)           # chunks 0-3 all transposed now
        if nb > 1:
            emit_A_quad(1, 0)
        emit_xw_chunk(3)
        gts[5] = load_g(5)          # slot of wih2 (PE-freed by xw2)
        gts[6] = load_g(6)          # slot of wih3 (PE-freed by xw3)
        whh_done = 0
        for tg4 in range(4, nt // 4):
            emit_g(tg4, gts.pop(tg4))
            if tg4 + 3 < nt // 4:
                gts[tg4 + 3] = load_g(tg4 + 3)   # slot of g(tg4): after its cast
            # thread W_hh transposes through the g loop (2 per group)
            for _ in range(2):
                if whh_done < 16:
                    emit_whh_chunk(whh_done)
                    whh_done += 1
            if tg4 % 4 == 3:
                cq = tg4 // 4
                emit_A_quad(0, cq)
                if nb > 1:
                    emit_A_quad(1, cq)
        while whh_done < 16:
            emit_whh_chunk(whh_done)
            whh_done += 1

        # ---------------- K steps ----------------
        for k in range(k_steps):
            # schedule: A(0) A(1) B(0) A(2) B(1) A(3) B(2) B(3)
            # (step 0's A(0)/A(1) were emitted inside the prolog above)
            if k > 0:
                emit_A(0)
                emit_fin(0)
                if nb > 1:
                    emit_A(1)
                    emit_fin(1)
            else:
                emit_fin(0)
                if nb > 1:
                    emit_fin(1)
            emit_B(0, k)
            for j in range(2, nb):
                emit_A(j)
                emit_fin(j)
                emit_B(j - 1, k)
            if nb > 1:
                emit_B(nb - 1, k)

    return nc


_NC_CACHE = {}


def _get_nc():
    if "full" not in _NC_CACHE:
        nc = build_bass()
        nc.finalize()
        _NC_CACHE["full"] = nc
    return _NC_CACHE["full"]


def kernel(f_x, g_S, W_ih, W_hh, b_ih, b_hh):
    from concourse.bass_utils import run_bass_kernel_spmd

    nc = _get_nc()
    f_x = np.ascontiguousarray(f_x, dtype=np.float32)
    g_S = np.ascontiguousarray(g_S, dtype=np.float32)
    W_ih = np.ascontiguousarray(W_ih, dtype=np.float32)
    W_hh = np.ascontiguousarray(W_hh, dtype=np.float32)
    b_ih = np.ascontiguousarray(b_ih, dtype=np.float32)
    b_hh = np.ascontiguousarray(b_hh, dtype=np.float32)
    in_maps = [
        {
            "f_x": f_x[c * B_LOC:(c + 1) * B_LOC],
            "g_S": g_S,
            "W_ih": W_ih,
            "W_hh": W_hh,
            "b_ih": b_ih,
            "b_hh": b_hh,
        }
        for c in range(N_CORES)
    ]
    res = run_bass_kernel_spmd(nc, in_maps, core_ids=list(range(N_CORES)))
    return np.concatenate([res.results[c]["out"] for c in range(N_CORES)], axis=0)


if __name__ == "__main__":
    nc = build_bass()
    nc.finalize()
    print("built ok")


# revision 27
# speedup vs baseline: 1.0437x; 1.0437x over previous
"""Trainium2 Bass kernel for nn_AttLSTM (attention-LSTM, K=4 steps).

Math per step (reference):
    a = softmax(h @ g_S.T, axis=1)            # [B, S]
    r = a @ g_S                               # [B, D]
    gates = f_x @ W_ih.T + b_ih + [h, r] @ W_hh.T + b_hh
    i, f, g, o = split(gates, 4)
    c' = sig(f)*c + sig(i)*tanh(g); h' = sig(o)*tanh(c') + f_x

Design (per core, data-parallel over batch: B_loc = 512 rows/core):
  - logits matmuls in fp16 (fp8 logits distort the peaked softmax too
    much: measured 4e-2 rel err vs the 2e-2 gate).  The attention
    readout r = p @ g AND the gates matmul run in fp8e4 with
    MatmulPerfMode.DoubleRow (2 k-tiles per instruction, ~1.7x PE rate;
    emulated rel err 1.4e-2 < 2e-2).
  - x @ W_ih.T + biases precomputed once (x == f_x every step) -> xw
    in fp16 (fp8 xw eats too much of the error budget).
  - g_S kept two ways, both RESIDENT in SBUF (no per-step DRAM
    streaming): transposed [D, S] fp16 (g_T, rhs of the logits matmul)
    and natural [S, D] fp8 (g8, rhs of the DoubleRow readout).
  - ALL transposes via PE transpose-mode in groups of [128,128] blocks
    into one PSUM bank + one strided copy back to SBUF.
  - softmax per 128-row b-tile: per-512-chunk negated max (DVE, from
    PSUM), exp with per-chunk bias straight from PSUM (ACT, fp8 out) +
    accum_out row-sums.  The global per-row rescale p *= exp(m_chunk -
    m_row) is folded INTO the p-transpose: the transpose matmul's
    "identity" operand is replaced by diag(fcorr) per chunk, so the
    rescale costs zero extra DVE/ACT passes.  The denominator uses the
    same fp8-quantized fcorr for consistency.
  - logits matmuls emitted in PSUM-bank *quads* with the contraction
    loop outermost so 4 consecutive instructions share lhsT.
  - prolog: W_ih/W_hh/f_x loaded raw f32 on the sync+scalar HW DMA
    queues (parallel rings), g_S cast-loaded f32->f16 on the gpsimd SW
    queues; gpsimd casts staging tiles to f16/fp8 SBUF-to-SBUF.  The
    step-0 A-phase of b-tiles 0/1 is threaded between the g_S group
    transposes so the PE never idles waiting for DMA.
  - sigmoid computed as 0.5*tanh(x/2)+0.5 so the single `exp_and_others`
    ACT table set (Exp + Tanh) serves the whole kernel.
  - LSTM pointwise math split between DVE and gpsimd (gpsimd is
    otherwise idle), carrying z = 2c as state.
"""

import os
import sys

import numpy as np

for _p in ("/opt/trn_rl_repo",):
    if _p not in sys.path and os.path.isdir(_p):
        sys.path.insert(0, _p)

# Problem sizes (hardcoded per spec).
B, S, D = 4096, 8192, 512
H = D
N_CORES = 8
B_LOC = B // N_CORES          # 512 rows per core
K_STEPS = 4
P = 128                       # partitions


def build_bass(b_loc=B_LOC, s=S, k_steps=K_STEPS):
    import concourse.mybir as mybir
    import concourse.tile as tile
    from concourse import bacc
    from concourse.masks import make_identity
    from contextlib import ExitStack

    f32 = mybir.dt.float32
    f16 = mybir.dt.float16
    f8 = mybir.dt.float8e4
    AF = mybir.ActivationFunctionType
    ALU = mybir.AluOpType
    AX = mybir.AxisListType
    DR = mybir.MatmulPerfMode.DoubleRow

    nb = b_loc // P               # b-tiles per core
    nd = D // P                   # contraction chunks over D
    ns = s // 512                 # s-chunks of 512
    ns2 = s // 1024               # softmax pair-chunks of 1024
    nt = s // P                   # s-tiles of 128
    ng = (4 * H) // 512           # gate chunks

    nc = bacc.Bacc("TRN2", target_bir_lowering=False, debug=False)

    f_x = nc.dram_tensor("f_x", [b_loc, D], f32, kind="ExternalInput")
    g_S = nc.dram_tensor("g_S", [s, D], f32, kind="ExternalInput")
    W_ih = nc.dram_tensor("W_ih", [4 * H, D], f32, kind="ExternalInput")
    W_hh = nc.dram_tensor("W_hh", [4 * H, 2 * H], f32, kind="ExternalInput")
    b_ih = nc.dram_tensor("b_ih", [4 * H], f32, kind="ExternalInput")
    b_hh = nc.dram_tensor("b_hh", [4 * H], f32, kind="ExternalInput")
    out = nc.dram_tensor("out", [b_loc, D], f32, kind="ExternalOutput")

    with tile.TileContext(nc) as tc, ExitStack() as ctx:
        const = ctx.enter_context(tc.tile_pool(name="const", bufs=1))
        g_T = const.tile([P, nd, s], f16)            # g_S.T resident fp16
        g8 = const.tile([P, nt, D], f8)              # g_S natural resident fp8
        whhT = const.tile([P, 2 * nd, 4 * H], f8)    # W_hh.T resident fp8
        xw = const.tile([P, nb, 4 * H], f16)         # f_x@W_ih.T + biases
        fx16 = const.tile([P, nb, D], f16)
        br16 = const.tile([1, 4 * H], f16)
        ones16 = const.tile([1, P], f16)
        ident = const.tile([P, P], f16)
        ident8 = const.tile([P, P], f8)

        # staging pool for raw f32 W chunks + f16 g groups
        wst_pool = ctx.enter_context(tc.tile_pool(name="wst", bufs=2))
        w16_pool = ctx.enter_context(tc.tile_pool(name="w16", bufs=3))
        wih_pool = ctx.enter_context(tc.tile_pool(name="wih", bufs=1))
        p_pool = ctx.enter_context(tc.tile_pool(name="p_pool", bufs=2))
        pt_pool = ctx.enter_context(tc.tile_pool(name="ptp", bufs=3))
        ht_pool = ctx.enter_context(tc.tile_pool(name="htp", bufs=4))
        h8_pool = ctx.enter_context(tc.tile_pool(name="h8p", bufs=4))
        rt_pool = ctx.enter_context(tc.tile_pool(name="rtp", bufs=1))
        rh_pool = ctx.enter_context(tc.tile_pool(name="rhp", bufs=2))
        lstm_pool = ctx.enter_context(tc.tile_pool(name="lstm", bufs=2))
        z_pool = ctx.enter_context(tc.tile_pool(name="zp", bufs=4))
        st_pool = ctx.enter_context(tc.tile_pool(name="stp", bufs=2))
        dg_pool = ctx.enter_context(tc.tile_pool(name="dgp", bufs=8))

        ps_log = ctx.enter_context(tc.tile_pool(name="ps_log", bufs=2, space="PSUM"))
        ps_g = ctx.enter_context(tc.tile_pool(name="ps_g", bufs=2, space="PSUM"))
        ps_tp = ctx.enter_context(tc.tile_pool(name="ps_tp", bufs=2, space="PSUM"))

        make_identity(nc, ident[:])
        make_identity(nc, ident8[:])

        _tpn = [0]

        def tp_group(blocks, dst, copy_engine="v", dst2=None,
                     copy2_engine="s"):
            """PE-transpose len(blocks) [128,128] fp16 blocks into one
            PSUM group tile, then one (possibly strided) copy into dst
            (shape [P, len(blocks), P]).  dst2 (if given) gets a second
            copy (e.g. fp8 cast of an fp16 transpose)."""
            n = len(blocks)
            _tpn[0] += 1
            tp = ps_tp.tile([P, n, P], f16, tag="tp", name=f"tp_{_tpn[0]}")
            for t, blk in enumerate(blocks):
                nc.tensor.transpose(tp[:, t, :], blk, ident[:])
            src = tp[:]
            if copy_engine == "v":
                nc.vector.tensor_copy(dst, src)
            else:
                nc.scalar.copy(dst, src)
            if dst2 is not None:
                if copy2_engine == "v":
                    nc.vector.tensor_copy(dst2, src)
                else:
                    nc.scalar.copy(dst2, src)

        # ---------------- prolog DMA kickoff ----------------
        nc.vector.memset(ones16[:], 1.0)

        nc.gpsimd.dma_start(br16[:], b_ih[:].rearrange("(a n) -> a n", a=1))
        nc.gpsimd.dma_start(br16[:], b_hh[:].rearrange("(a n) -> a n", a=1),
                            accum_op=ALU.add)

        # f_x cast-loaded f32->f16 on gpsimd (first in its queue)
        for j in range(nb):
            nc.gpsimd.dma_start(fx16[:, j, :], f_x[j * P:(j + 1) * P, :])

        # W_ih cast-loaded f32->f16 on gpsimd (chunk = 4 row-tiles =
        # one 512-wide gate-column chunk)
        wih16 = {}

        def load_wih(cchunk):
            t = w16_pool.tile([P, 4, D], f16, tag="g16", name=f"wih16_{cchunk}")
            nc.gpsimd.dma_start(
                t[:], W_ih[cchunk * 4 * P:(cchunk + 1) * 4 * P, :].rearrange(
                    "(a p) d -> p a d", p=P))
            wih16[cchunk] = t


        # W_hh raw f32 chunks on sync HW queues (chunk = 1 row-tile);
        # nothing else ever waits on the sync queue, so the staged-slot
        # reuse deps cannot deadlock other engines.
        whh_st = {}
        for hchunk in range(16):
            t = wst_pool.tile([P, 1, 2 * H], f32, tag="wst",
                              name=f"whhst_{hchunk}")
            nc.sync.dma_start(
                t[:], W_hh[hchunk * P:(hchunk + 1) * P, :].rearrange(
                    "(a p) d -> p a d", p=P))
            whh_st[hchunk] = t

        # g_S cast-loads f32->f16 on gpsimd SW queues (group = 4 s-tiles)
        def load_g(tg4):
            gt = w16_pool.tile([P, 4, D], f16, tag="g16", name=f"gload_{tg4}")
            nc.gpsimd.dma_start(
                gt[:], g_S[tg4 * 4 * P:(tg4 + 1) * 4 * P, :].rearrange(
                    "(a p) d -> p a d", p=P))
            return gt

        # ---------------- prolog compute helpers ----------------
        # f_x transposes: fp16 for logits lhsT + fp8 for gates lhsT
        hT, hT8 = {}, {}

        def emit_fxT():
            for j in range(nb):
                t = ht_pool.tile([P, nd, P], f16, tag="hT", name=f"fxT_{j}")
                t8 = h8_pool.tile([P, nd, P], f8, tag="hT8", name=f"fxT8_{j}")
                tp_group([fx16[:, j, kk * P:(kk + 1) * P] for kk in range(nd)],
                         t[:], copy_engine="v" if j % 2 == 0 else "s",
                         dst2=t8[:], copy2_engine="s" if j % 2 == 0 else "v")
                hT[j] = t
                hT8[j] = t8

        def emit_xw_chunk(cchunk):
            """W_ih rows [512c, 512(c+1)) -> wihT columns; then xw chunk c
            for all b-tiles.  Chunk c of xw only needs those columns."""
            w16 = wih16.pop(cchunk)
            wihT = wih_pool.tile([P, nd, 4 * P], f16, tag="wih",
                                 name=f"wihT_{cchunk}")
            for a in range(4):
                tp_group([w16[:, a, kk * P:(kk + 1) * P] for kk in range(nd)],
                         wihT[:, :, a * P:(a + 1) * P],
                         copy_engine="v" if a % 2 == 0 else "s")
            for j in range(nb):
                gp = ps_g.tile([P, 512], f32, tag="psg", name=f"xwps_{j}_{cchunk}")
                nc.tensor.matmul(gp[:], ones16[:],
                                 br16[:, cchunk * 512:(cchunk + 1) * 512],
                                 start=True, stop=False)
                for kk in range(nd):
                    nc.tensor.matmul(gp[:], hT[j][:, kk, :], wihT[:, kk, :],
                                     start=False, stop=(kk == nd - 1))
                nc.vector.tensor_copy(xw[:, j, cchunk * 512:(cchunk + 1) * 512],
                                      gp[:])

        def emit_whh_chunk(hchunk):
            """W_hh row-tile -> whhT fp8 columns [128h, 128(h+1))."""
            st = whh_st.pop(hchunk)
            w16 = w16_pool.tile([P, 1, 2 * H], f16, tag="wh16",
                                name=f"whh16_{hchunk}")
            if hchunk % 2 == 0:
                nc.scalar.copy(w16[:], st[:])
            else:
                nc.vector.tensor_copy(w16[:], st[:])
            tp_group([w16[:, 0, kk * P:(kk + 1) * P] for kk in range(2 * nd)],
                     whhT[:, :, hchunk * P:(hchunk + 1) * P],
                     copy_engine="v" if hchunk % 2 == 0 else "s")
            del st

        def emit_g(tg4, gt):
            # fp8 natural copy on DVE (cast-DMAs would contend with the
            # g load stream on the DMA rings)
            nc.vector.tensor_copy(g8[:, tg4 * 4:(tg4 + 1) * 4, :], gt[:])
            for a in range(4):
                t = tg4 * 4 + a
                tp_group([gt[:, a, kk * P:(kk + 1) * P] for kk in range(nd)],
                         g_T[:, :, t * P:(t + 1) * P],
                         copy_engine="v" if t % 2 == 0 else "s")

        # ---------------- step state ----------------
        z = {}
        for j in range(nb):
            zt = z_pool.tile([P, D], f16, tag="z", name=f"z0_{j}")
            nc.vector.memset(zt[:], 0.0)
            z[j] = zt

        pbuf, negmaxes, sums, fcorr, rsum = {}, {}, {}, {}, {}

        def alloc_A(j):
            pbuf[j] = p_pool.tile([P, s], f8, tag="p", name=f"p_{j}")
            negmaxes[j] = st_pool.tile([P, ns2], f32, tag="nmx", name=f"nmx_{j}")
            sums[j] = st_pool.tile([P, ns2], f32, tag="sums", name=f"sums_{j}")

        def emit_A_pair(j, pc):
            """logits + negmax + exp for the 1024-wide pair-chunk pc of
            b-tile j.  One [P,1024] psl tile spans 2 PSUM banks; the
            contraction loop is outermost so 2 consecutive matmuls share
            lhsT.  With bufs=2, pair pc+1's matmuls fully hide the exp of
            pair pc (which is the stats at 1024 granularity: half the
            DVE/ACT instruction count of 512-chunks)."""
            ps = ps_log.tile([P, 1024], f32, tag="psl", name=f"psl_{j}_{pc}")
            for kk in range(nd):
                for u in range(2):
                    nc.tensor.matmul(
                        ps[:, u * 512:(u + 1) * 512], hT[j][:, kk, :],
                        g_T[:, kk, (2 * pc + u) * 512:(2 * pc + u + 1) * 512],
                        start=(kk == 0), stop=(kk == nd - 1))
            nc.vector.tensor_reduce(
                negmaxes[j][:, pc:pc + 1], ps[:],
                axis=AX.X, op=ALU.max, negate=True)
            nc.scalar.activation(
                pbuf[j][:, pc * 1024:(pc + 1) * 1024], ps[:],
                AF.Exp, bias=negmaxes[j][:, pc:pc + 1],
                accum_out=sums[j][:, pc:pc + 1])

        def emit_A_quad(j, cq):
            emit_A_pair(j, 2 * cq)
            emit_A_pair(j, 2 * cq + 1)

        def emit_A(j):
            alloc_A(j)
            for pc in range(ns2):
                emit_A_pair(j, pc)

        def emit_fin(j):
            """global max, correction factors (quantized fp8 for the
            diag fold), 1/sum for b-tile j"""
            nm = st_pool.tile([P, 1], f32, tag="nm", name=f"nm_{j}")
            nc.vector.tensor_reduce(nm[:], negmaxes[j][:], axis=AX.X, op=ALU.min)
            delta = st_pool.tile([P, ns2], f32, tag="delta", name=f"delta_{j}")
            # delta_i = m_i - m = -negmax_i + nm
            nc.vector.tensor_scalar(delta[:], negmaxes[j][:], -1.0, nm[:],
                                    op0=ALU.mult, op1=ALU.add)
            fc = st_pool.tile([P, ns2], f32, tag="fc", name=f"fc_{j}")
            nc.scalar.activation(fc[:], delta[:], AF.Exp)
            fcorr[j] = fc
            # quantize fcorr exactly as the diag-fold will (f32 -> fp8 RNE)
            fc8 = st_pool.tile([P, ns2], f8, tag="fc8", name=f"fc8_{j}")
            nc.vector.tensor_copy(fc8[:], fc[:])
            fc8f = st_pool.tile([P, ns2], f32, tag="fc8f", name=f"fc8f_{j}")
            nc.scalar.copy(fc8f[:], fc8[:])
            ws = st_pool.tile([P, ns2], f32, tag="ws", name=f"ws_{j}")
            nc.vector.scalar_tensor_tensor(ws[:], sums[j][:], 0.0, fc8f[:],
                                           op0=ALU.add, op1=ALU.mult)
            ssum = st_pool.tile([P, 1], f32, tag="ssum", name=f"ssum_{j}")
            nc.vector.tensor_reduce(ssum[:], ws[:], axis=AX.X, op=ALU.add)
            rs = st_pool.tile([P, 1], f32, tag="rs", name=f"rs_{j}")
            nc.vector.reciprocal(rs[:], ssum[:])
            rsum[j] = rs

        def emit_B(j, k):
            """transpose p (with the fcorr diag fold), fp8 DR readout,
            fp8 DR gates, LSTM update"""
            rp = ps_g.tile([P, D], f32, tag="psg", name=f"psr_{j}")
            pTg = {}
            diags = {}

            def mk_diag(pc):
                dg = dg_pool.tile([P, P], f8, tag="dg", name=f"dg_{j}_{pc}")
                if pc % 2 == 0:
                    nc.vector.tensor_scalar_mul(dg[:], ident8[:],
                                                fcorr[j][:, pc:pc + 1])
                else:
                    nc.scalar.mul(dg[:], ident8[:], fcorr[j][:, pc:pc + 1])
                diags[pc] = dg

            def tpg(ig):
                """transpose-and-rescale: regular matmul p_blk.T @ diag(fc)
                (transpose-mode requires a permutation rhs, a plain matmul
                does not).  LDWEIGHTS = p block, rhs = diag, N=128."""
                grp = pt_pool.tile([P, 4, P], f8, tag="pt", name=f"pt_{j}_{ig}")
                _tpn[0] += 1
                tp = ps_tp.tile([P, 4, P], f32, tag="tp",
                                name=f"ptp_{_tpn[0]}")
                dg = diags.pop(ig // 2) if ig % 2 == 1 else diags[ig // 2]
                for t in range(4):
                    nc.tensor.matmul(
                        tp[:, t, :],
                        pbuf[j][:, (ig * 4 + t) * P:(ig * 4 + t + 1) * P],
                        dg[:], start=True, stop=True)
                if ig % 2 == 0:
                    nc.vector.tensor_copy(grp[:], tp[:])
                else:
                    nc.scalar.copy(grp[:], tp[:])
                pTg[ig] = grp

            mk_diag(0)
            mk_diag(1)
            mk_diag(2)
            tpg(0)
            tpg(1)
            for ig in range(nt // 4):
                if ig % 2 == 0 and ig // 2 + 3 < ns2:
                    mk_diag(ig // 2 + 3)
                if ig + 2 < nt // 4:
                    tpg(ig + 2)
                for u in range(2):
                    c = ig * 2 + u
                    nc.tensor.matmul(rp[:], pTg[ig][:, 2 * u:2 * u + 2, :],
                                     g8[:, ig * 4 + 2 * u:ig * 4 + 2 * u + 2, :],
                                     start=(c == 0), stop=(c == nt // 2 - 1),
                                     perf_mode=DR)
                del pTg[ig]
            r16 = rh_pool.tile([P, D], f16, tag="r16", bufs=1, name=f"r16_{j}")
            nc.vector.tensor_scalar_mul(r16[:], rp[:], rsum[j][:])
            rT8 = rt_pool.tile([P, nd, P], f8, tag="rT", name=f"rT_{j}")
            tp_group([r16[:, kk * P:(kk + 1) * P] for kk in range(nd)], rT8[:])
            # gates = xw + h@Whh_h.T + r@Whh_r.T, fp8 DoubleRow, in
            # PSUM-bank pairs with the contraction loop outermost
            tt = [None] * ng
            for half in range(2):
                gp = [ps_g.tile([P, 512], f32, tag="psg",
                                name=f"psg_{j}_{half}_{u}") for u in range(2)]
                for q in range(nd // 2):
                    for u in range(2):
                        n = half * 2 + u
                        nc.tensor.matmul(
                            gp[u][:], hT8[j][:, 2 * q:2 * q + 2, :],
                            whhT[:, 2 * q:2 * q + 2, n * 512:(n + 1) * 512],
                            start=(q == 0), stop=False, perf_mode=DR)
                for q in range(nd // 2):
                    for u in range(2):
                        n = half * 2 + u
                        nc.tensor.matmul(
                            gp[u][:], rT8[:, 2 * q:2 * q + 2, :],
                            whhT[:, nd + 2 * q:nd + 2 * q + 2,
                                 n * 512:(n + 1) * 512],
                            start=False, stop=(q == nd // 2 - 1), perf_mode=DR)
                for u in range(2):
                    n = half * 2 + u
                    pre = lstm_pool.tile([P, 512], f16, tag="pre", bufs=1,
                                         name=f"pre_{j}_{n}")
                    nc.vector.scalar_tensor_tensor(
                        pre[:], gp[u][:], 0.0, xw[:, j, n * 512:(n + 1) * 512],
                        op0=ALU.add, op1=ALU.add)
                    t = lstm_pool.tile([P, 512], f16, tag=f"t{n}", bufs=1,
                                       name=f"t{n}_{j}")
                    # i,f,o gates: tanh(x/2) (-> sigmoid); g gate: tanh(x)
                    nc.scalar.activation(t[:], pre[:], AF.Tanh,
                                         scale=1.0 if n == 2 else 0.5)
                    tt[n] = t
            ti, tf, tg, to = tt
            # z' = 0.5*(tf+1)*z + (ti+1)*tg       (z = 2c)
            v = lstm_pool.tile([P, D], f16, tag="v", bufs=1, name=f"v_{j}")
            nc.vector.scalar_tensor_tensor(v[:], ti[:], 1.0, tg[:],
                                           op0=ALU.add, op1=ALU.mult)
            q = lstm_pool.tile([P, D], f16, tag="q", bufs=1, name=f"q_{j}")
            nc.vector.scalar_tensor_tensor(q[:], tf[:], 1.0, z[j][:],
                                           op0=ALU.add, op1=ALU.mult)
            zn = z_pool.tile([P, D], f16, tag="z", name=f"z_{j}")
            nc.vector.scalar_tensor_tensor(zn[:], q[:], 0.5, v[:],
                                           op0=ALU.mult, op1=ALU.add)
            z[j] = zn
            # h' = 0.5*(to+1)*tanh(z'/2) + f_x
            y = lstm_pool.tile([P, D], f16, tag="y", bufs=1, name=f"y_{j}")
            nc.scalar.activation(y[:], zn[:], AF.Tanh, scale=0.5)
            w = lstm_pool.tile([P, D], f16, tag="w", bufs=1, name=f"w_{j}")
            nc.vector.scalar_tensor_tensor(w[:], to[:], 1.0, y[:],
                                           op0=ALU.add, op1=ALU.mult)
            if k < k_steps - 1:
                h16 = rh_pool.tile([P, D], f16, tag="h16", bufs=1, name=f"h16_{j}")
                nc.vector.scalar_tensor_tensor(h16[:], w[:], 0.5, fx16[:, j, :],
                                               op0=ALU.mult, op1=ALU.add)
                hTn = ht_pool.tile([P, nd, P], f16, tag="hT", name=f"hT_{j}")
                hTn8 = h8_pool.tile([P, nd, P], f8, tag="hT8", name=f"hT8_{j}")
                tp_group([h16[:, kk * P:(kk + 1) * P] for kk in range(nd)],
                         hTn[:], copy_engine="v" if j % 2 == 0 else "s",
                         dst2=hTn8[:], copy2_engine="s" if j % 2 == 0 else "v")
                hT[j] = hTn
                hT8[j] = hTn8
            else:
                ho = z_pool.tile([P, D], f16, tag="z", name=f"ho_{j}")
                nc.vector.scalar_tensor_tensor(ho[:], w[:], 0.5, fx16[:, j, :],
                                               op0=ALU.mult, op1=ALU.add)
                nc.gpsimd.dma_start(out[j * P:(j + 1) * P, :], ho[:])

        # ---------------- interleaved prolog + step-0 A(0)/A(1) --------
        # gpsimd FIFO rule: a load that reuses a g16-ring slot freed by a
        # gpsimd cast must be emitted AFTER that cast (FIFO self-waits
        # deadlock otherwise).  Ring (bufs=3) allocation order:
        # wih0 wih1 g0 | g1 g2 g3 | wih2 wih3 g4 | g5 g6 g7 | g8 ...
        load_wih(0)
        load_wih(1)
        gts = {0: load_g(0)}
        emit_fxT()
        emit_xw_chunk(0)            # frees wih0 (PE-side)
        gts[1] = load_g(1)          # slot of wih0
        emit_xw_chunk(1)            # frees wih1
        gts[2] = load_g(2)          # slot of wih1
        # pbufs share p_pool slots; allocate after the early prolog tiles
        alloc_A(0)
        if nb > 1:
            alloc_A(1)
        emit_g(0, gts.pop(0))
        gts[3] = load_g(3)          # slot of g0: after cast g0
        emit_g(1, gts.pop(1))
        load_wih(2)                 # slot of g1: after cast g1
        emit_g(2, gts.pop(2))
        load_wih(3)                 # slot of g2: after cast g2
        emit_xw_chunk(2)
        emit_g(3, gts.pop(3))
        gts[4] = load_g(4)          # slot of g3: after cast g3
        emit_A_quad(0, 0)           # chunks 0-3 all transposed now
        if nb > 1:
            emit_A_quad(1, 0)
        emit_xw_chunk(3)
        gts[5] = load_g(5)          # slot of wih2 (PE-freed by xw2)
        gts[6] = load_g(6)          # slot of wih3 (PE-freed by xw3)
        whh_done = 0
        for tg4 in range(4, nt // 4):
            emit_g(tg4, gts.pop(tg4))
            if tg4 + 3 < nt // 4:
                gts[tg4 + 3] = load_g(tg4 + 3)   # slot of g(tg4): after its cast
            # thread W_hh transposes through the g loop (2 per group)
            for _ in range(2):
                if whh_done < 16:
                    emit_whh_chunk(whh_done)
                    whh_done += 1
            if tg4 % 4 == 3:
                cq = tg4 // 4
                emit_A_quad(0, cq)
                if nb > 1:
                    emit_A_quad(1, cq)
        while whh_done < 16:
            emit_whh_chunk(whh_done)
            whh_done += 1

        # ---------------- K steps ----------------
        for k in range(k_steps):
            # schedule: A(0) A(1) B(0) A(2) B(1) A(3) B(2) B(3)
            # (step 0's A(0)/A(1) were emitted inside the prolog above)
            if k > 0:
                emit_A(0)
                emit_fin(0)
                if nb > 1:
                    emit_A(1)
                    emit_fin(1)
            else:
                emit_fin(0)
                if nb > 1:
                    emit_fin(1)
            emit_B(0, k)
            for j in range(2, nb):
                emit_A(j)
                emit_fin(j)
                emit_B(j - 1, k)
            if nb > 1:
                emit_B(nb - 1, k)

    return nc


_NC_CACHE = {}


def _get_nc():
    if "full" not in _NC_CACHE:
        nc = build_bass()
        nc.finalize()
        _NC_CACHE["full"] = nc
    return _NC_CACHE["full"]


def kernel(f_x, g_S, W_ih, W_hh, b_ih, b_hh):
    from concourse.bass_utils import run_bass_kernel_spmd

    nc = _get_nc()
    f_x = np.ascontiguousarray(f_x, dtype=np.float32)
    g_S = np.ascontiguousarray(g_S, dtype=np.float32)
    W_ih = np.ascontiguousarray(W_ih, dtype=np.float32)
    W_hh = np.ascontiguousarray(W_hh, dtype=np.float32)
    b_ih = np.ascontiguousarray(b_ih, dtype=np.float32)
    b_hh = np.ascontiguousarray(b_hh, dtype=np.float32)
    in_maps = [
        {
            "f_x": f_x[c * B_LOC:(c + 1) * B_LOC],
            "g_S": g_S,
            "W_ih": W_ih,
            "W_hh": W_hh,
            "b_ih": b_ih,
            "b_hh": b_hh,
        }
        for c in range(N_CORES)
    ]
    res = run_bass_kernel_spmd(nc, in_maps, core_ids=list(range(N_CORES)))
    return np.concatenate([res.results[c]["out"] for c in range(N_CORES)], axis=0)


if __name__ == "__main__":
    nc = build_bass()
    nc.finalize()
    print("built ok")
